# revision 26
# baseline (speedup 1.0000x reference)
"""Multi-head attention forward on 8 Trainium2 NeuronCores (Bass/Tile).

Problem: nn_MultiHeadAttention — B=8, T=1024, C=768, H=12, D=64, fp32.

Sharding: data-parallel over batch — B=8 -> one batch element per core; weights
broadcast to all cores. No collectives. Host pre-transposes x[b] to x^T [C, T],
casts x and all weights to bf16, and pre-arranges biases; the full output is
gathered by stacking per-core results and upcasting bf16 -> f32.

All matmul operands are bf16 (1 col/cycle warm — same streaming rate as f32r
at N>=256 — at half the DMA/SBUF footprint; measured faster than f32r on HW:
bf16 FWL loads weights 2 elems/cycle). Matmul outputs are <=512 f32 columns
(one PSUM bank) — wider outputs fail the s3d3 ISA check. End-to-end rel err
~6.5e-3 against the fp32 reference (budget 2e-2), dominated by bf16 rounding
of Q/K/V/P.

Per-core structure:
  1. V = x @ Wv + bv -> V_aug [128, T/128, H, 65] bf16 with a ones column per
     head: the ones row of the attention-weighted product yields the softmax
     denominator for free. V psums draw from the psY pool, which is idle
     across the rep boundary, so rep i+1's V matmuls never queue behind rep
     i's output-projection evacuations (which own the psS pool's FIFO).
  2. Per head pair p: Q^T chunk [128, T] via matmul(lhsT=Wq[:,co], rhs=xT)
     (head-major transposed — exactly the S^T rhs), K^T as two zero-padded
     tiles KTz[hh] [128, T] (the other head's 64 partitions zeroed once per
     rep; contracting the full 128 partitions with an unpadded Q^T rhs nulls
     the other head's contribution). Pair 0's projection psums also draw
     from psY (same rep-boundary argument).
  3. Attention is one software-pipelined emission stream: for each S item
     (h, j) the S^T psum [128, 1024] (two 512-wide matmuls), then exp on
     ScalarE ([128,1024] psum -> bf16 sbuf; no max subtraction: |S|/8 is
     small, exp is exact enough). The av accumulation
     psy[h][0:65] += matmul(lhsT=V_aug[:, j, h, :], rhs=P-half) TRAILS the S
     stream by AV_LEAD=2 S-items, crossing head boundaries: the PE never
     parks on the exp that feeds the next av, and the ScalarE queue always
     has S^T backlog. Projection blocks for the next pair land mid-head
     (after j=4) where the exp backlog covers them.
  4. Normalize y^T = Ytil[0:64] * recip(Ytil[64]): DVE copies the psum
     denominator row to SBUF (DVE reciprocal reading PSUM directly returns
     garbage on HW), reciprocal_approx_fast, GpSimd partition_broadcast, DVE
     multiply into Y^T [C, T] bf16. psY bufs=2 lets head h+1 accumulate
     while head h normalizes.
  5. out[t, :] = matmul(lhsT=YT[:, k, t128], rhs=Wp[:, k, :]) + bp -> DMA
     (bf16; host upcasts). y stores issue from the ACT engine's DMA queue:
     on SP they would park at the queue head waiting on the out tile,
     stalling the next rep's input loads queued behind them.

PSUM: psS [128,1024] f32 bufs=2 (4 banks) rotates S^T tiles and the
steady-state projection/output psums; psY [128,1024] f32 bufs=2 (4 banks)
holds the attention accumulators plus the V-phase/pair-0 psums. Wv and Wp
rotate through one bufs=2 tag: each rep's Wv load waits only on the previous
rep's V phase, not on Wp's output-projection reads, so input DMA for rep i+1
fully overlaps rep i's attention tail.

Measured (slope of reps=257 vs reps=1 builds, 8 cores): ~197 us/rep,
rel err 6.5e-3. TimelineSim steady-state marginal: ~168 us/rep.
"""
import numpy as np

B, T, C = 8, 1024, 768
H, D = 12, 64
P = 128
KS = C // P          # 6 contraction subtiles
TS = T // P          # 8 t subtiles
NI = T // 512        # 2 i-halves of 512
N_CORES = 8

_RUNNER_CACHE = {}


def build_nc(reps: int = 1, phases: int = 4, variant: str = "full"):
    import concourse.bacc as bacc
    import concourse.mybir as mybir
    import concourse.tile as tile
    from contextlib import ExitStack

    f32 = mybir.dt.float32
    bf16 = mybir.dt.bfloat16
    AF = mybir.ActivationFunctionType
    ALU = mybir.AluOpType

    nc = bacc.Bacc(num_devices=N_CORES)

    xT_d = nc.dram_tensor("xT", [C, T], bf16, kind="ExternalInput")
    W_d = {w: nc.dram_tensor(f"W{w}", [C, C], bf16, kind="ExternalInput")
           for w in ("q", "k", "v", "p")}
    bqT_d = nc.dram_tensor("bqT", [P, KS], f32, kind="ExternalInput")
    bkT_d = nc.dram_tensor("bkT", [P, KS], f32, kind="ExternalInput")
    bvB_d = nc.dram_tensor("bvB", [P, C], f32, kind="ExternalInput")
    bpB_d = nc.dram_tensor("bpB", [P, C], f32, kind="ExternalInput")
    y_d = nc.dram_tensor("y", [T, C], bf16, kind="ExternalOutput")

    with tile.TileContext(nc) as tc, ExitStack() as ctx:
        const = ctx.enter_context(tc.tile_pool(name="const", bufs=1))
        wvp = ctx.enter_context(tc.tile_pool(name="wvp", bufs=2))
        ppool = ctx.enter_context(tc.tile_pool(name="pt", bufs=6))
        npool = ctx.enter_context(tc.tile_pool(name="norm", bufs=2))
        opool = ctx.enter_context(tc.tile_pool(name="out", bufs=2))
        psS = ctx.enter_context(tc.tile_pool(name="psS", bufs=2, space="PSUM"))
        psY = ctx.enter_context(tc.tile_pool(name="psY", bufs=2, space="PSUM"))

        def body(_iv=None):
            AV_LEAD = 2
            PROJ_AT = 5
            import re as _re
            _m = _re.search(r"L(\d+)P(\d+)", variant)
            if _m:
                AV_LEAD, PROJ_AT = int(_m.group(1)), int(_m.group(2))

            # ---- loads ----
            xTr = const.tile([P, KS, T], bf16, tag="xT", name="xTr")
            Wr = {}
            for w in ("q", "k"):
                Wr[w] = const.tile([P, KS, C], bf16, tag=f"W{w}", name=f"W{w}r")
            Wr["v"] = wvp.tile([P, KS, C], bf16, tag="Wvp", name="Wvr")
            xT_r = xT_d.rearrange("(ks p) t -> p ks t", p=P)
            W_r = {w: W_d[w].rearrange("(ks p) c -> p ks c", p=P)
                   for w in ("q", "k", "v", "p")}
            # tiny bias loads FIRST so they don't queue behind the bulk loads
            bqT = const.tile([P, KS], f32, tag="bqT", name="bqT")
            nc.sync.dma_start(bqT[:], bqT_d[:, :])
            bkT = const.tile([P, KS], f32, tag="bkT", name="bkT")
            nc.sync.dma_start(bkT[:], bkT_d[:, :])
            bvB = const.tile([P, C], f32, tag="bvB", name="bvB")
            nc.sync.dma_start(bvB[:], bvB_d[:, :])
            bpB = const.tile([P, C], f32, tag="bpB", name="bpB")
            nc.sync.dma_start(bpB[:], bpB_d[:, :])
            # split per k-subtile, in consumption order (Wv+xT first)
            for k in range(KS):
                nc.sync.dma_start(Wr["v"][:, k, :], W_r["v"][:, k, :])
                nc.sync.dma_start(xTr[:, k, :], xT_r[:, k, :])
            for k in range(KS):
                nc.sync.dma_start(Wr["q"][:, k, :], W_r["q"][:, k, :])
            for k in range(KS):
                nc.sync.dma_start(Wr["k"][:, k, :], W_r["k"][:, k, :])
            ones1 = const.tile([P, 1], f32, tag="ones", name="ones1")
            nc.vector.memset(ones1[:], 1.0)

            # ---- V (natural layout) into V_aug with ones column ----
            V_aug = const.tile([P, TS, H, D + 1], bf16, tag="Vaug", name="Vaug")
            nc.vector.tensor_copy(V_aug[:, :, :, D:D + 1],
                                  ones1[:].to_broadcast([P, TS, H, 1]))
            for ts_ in range(TS):
                psv = psY.tile([P, 1024], f32, tag="psY", name="psv")
                for k in range(KS):
                    lhsT = xTr[:, k, ts_ * P:(ts_ + 1) * P]
                    nc.tensor.matmul(psv[:, 0:512], lhsT, Wr["v"][:, k, 0:512],
                                     start=(k == 0), stop=(k == KS - 1))
                    nc.tensor.matmul(psv[:, 512:768], lhsT, Wr["v"][:, k, 512:768],
                                     start=(k == 0), stop=(k == KS - 1))
                nc.vector.tensor_tensor(
                    V_aug[:, ts_, :, 0:D],
                    psv[:, 0:768].rearrange("p (h d) -> p h d", h=H),
                    bvB[:].rearrange("p (h d) -> p h d", h=H), op=ALU.add)

            # Wp: rotates into the second wvp slot; only waits on the
            # PREVIOUS rep's output projection, lands during attention
            Wr["p"] = wvp.tile([P, KS, C], bf16, tag="Wvp", name="Wpr")
            for k in range(KS):
                nc.sync.dma_start(Wr["p"][:, k, :], W_r["p"][:, k, :])

            if phases < 3:
                YTdummy = opool.tile([P, C], bf16, tag="ot", name="ytd")
                nc.vector.memset(YTdummy[:], 0.0)
                nc.sync.dma_start(y_d[0:P, :], YTdummy[:])
                return

            # ---- attention: software-pipelined S/exp/av stream ----
            YT = const.tile([P, KS, T], bf16, tag="YTs", name="YT")

            QTt = {}
            KTt = {}
            # zero-padded K^T tiles: head hh's tile has the other head's 64
            # partitions zeroed (once per rep — nothing else writes them)
            KTz = {}
            for parity in range(2):
                for hh in range(2):
                    KTz[(hh, parity)] = const.tile(
                        [P, T], bf16, tag=f"KTz{hh}_{parity}", name="KTz")
            nc.vector.memset(KTz[(0, 0)][64:128, :], 0.0)
            nc.vector.memset(KTz[(0, 1)][64:128, :], 0.0)
            nc.vector.memset(KTz[(1, 0)][0:64, :], 0.0)
            nc.vector.memset(KTz[(1, 1)][0:64, :], 0.0)

            def emit_qt(p, pool=None):
                QTt[p] = const.tile([P, T], bf16, tag=f"QT{p % 2}", name="QTp")
                pool = pool or psS
                psq = pool.tile([P, 1024], f32,
                                tag=("psY" if pool is psY else "psS"),
                                name="psq")
                for k in range(KS):
                    lhsT = Wr["q"][:, k, p * P:(p + 1) * P]
                    for i in range(NI):
                        nc.tensor.matmul(psq[:, i * 512:(i + 1) * 512], lhsT,
                                         xTr[:, k, i * 512:(i + 1) * 512],
                                         start=(k == 0), stop=(k == KS - 1))
                nc.vector.tensor_tensor(
                    QTt[p][:], psq[:],
                    bqT[:, p:p + 1].to_broadcast([P, T]), op=ALU.add)

            def emit_kt(p, pool=None):
                kt = (KTz[(0, p % 2)], KTz[(1, p % 2)])
                KTt[p] = kt
                pool = pool or psS
                psk = pool.tile([P, 1024], f32,
                                tag=("psY" if pool is psY else "psS"),
                                name="psk")
                for k in range(KS):
                    lhsT = Wr["k"][:, k, p * P:(p + 1) * P]
                    for i in range(NI):
                        nc.tensor.matmul(psk[:, i * 512:(i + 1) * 512], lhsT,
                                         xTr[:, k, i * 512:(i + 1) * 512],
                                         start=(k == 0), stop=(k == KS - 1))
                nc.vector.tensor_tensor(
                    kt[0][0:64, :], psk[0:64, :],
                    bkT[0:64, p:p + 1].to_broadcast([64, T]), op=ALU.add)
                nc.vector.tensor_tensor(
                    kt[1][64:128, :], psk[64:128, :],
                    bkT[64:128, p:p + 1].to_broadcast([64, T]), op=ALU.add)

            # pair 0's projections draw from psY: at the rep boundary the psS
            # FIFO is still owned by the previous rep's out-proj evacuations,
            # while psY's V slots free as soon as their evacuation lands
            emit_qt(0, pool=psY)
            emit_kt(0, pool=psY)

            # S-item stream; next pair's projection blocks land mid-head
            stream = []
            for p in range(KS):
                for hh in range(2):
                    for j in range(TS):
                        stream.append(("S", p, hh, j))
                        if j + 1 == min(PROJ_AT, TS) and p + 1 < KS:
                            stream.append(("QT", p + 1) if hh == 0
                                          else ("KT", p + 1))

            psy = {}
            pts = {}
            s_seq = []       # S items in emission order
            n_av = [0]       # next av to emit (index into s_seq)

            def emit_av(idx):
                p, hh, j = s_seq[idx]
                h = 2 * p + hh
                if h not in psy:
                    psy[h] = psY.tile([P, 1024], f32, tag="psY", name="psy")
                pt = pts.pop((h, j))
                for i in range(NI):
                    nc.tensor.matmul(
                        psy[h][0:D + 1, i * 512:(i + 1) * 512],
                        V_aug[:, j, h, :], pt[:, i * 512:(i + 1) * 512],
                        start=(j == 0), stop=(j == TS - 1))
                if j == TS - 1:
                    # normalize: y^T = Ytil[0:64] * recip(Ytil[64]); the
                    # denominator row stages through SBUF (DVE reciprocal
                    # reading PSUM directly returns garbage on HW)
                    ph_ = psy.pop(h)
                    dd = npool.tile([1, T], f32, tag="dd", name="dd")
                    nc.vector.tensor_copy(dd[0:1, :], ph_[D:D + 1, :])
                    rr = npool.tile([1, T], f32, tag="rr", name="rr")
                    nc.vector.reciprocal_approx_fast(rr[0:1, :], dd[0:1, :])
                    rb = npool.tile([D, T], f32, tag="rb", name="rb")
                    nc.gpsimd.partition_broadcast(rb[:], rr[0:1, :])
                    nc.vector.tensor_tensor(
                        YT[64 * hh:64 * hh + 64, p, :], ph_[0:D, :],
                        rb[:], op=ALU.mult)

            for item in stream:
                if item[0] == "S":
                    _, p, hh, j = item
                    h = 2 * p + hh
                    pss = psS.tile([P, 1024], f32, tag="psS", name="pss")
                    for i in range(NI):
                        nc.tensor.matmul(
                            pss[:, i * 512:(i + 1) * 512],
                            KTt[p][hh][:, j * P:(j + 1) * P],
                            QTt[p][:, i * 512:(i + 1) * 512],
                            start=True, stop=True)
                    pt = ppool.tile([P, 1024], bf16, tag="pt", name="pt")
                    nc.scalar.activation(pt[:], pss[:], AF.Exp, scale=0.125)
                    pts[(h, j)] = pt
                    s_seq.append((p, hh, j))
                elif item[0] == "QT":
                    emit_qt(item[1])
                else:
                    emit_kt(item[1])
                while n_av[0] < len(s_seq) - AV_LEAD:
                    emit_av(n_av[0])
                    n_av[0] += 1
            while n_av[0] < len(s_seq):
                emit_av(n_av[0])
                n_av[0] += 1

            if phases < 4:
                return

            # ---- output projection ----
            for ts_ in range(TS):
                po_ = psS.tile([P, 1024], f32, tag="psS", name="pso")
                for k in range(KS):
                    lhsT = YT[:, k, ts_ * P:(ts_ + 1) * P]
                    nc.tensor.matmul(po_[:, 0:512], lhsT, Wr["p"][:, k, 0:512],
                                     start=(k == 0), stop=(k == KS - 1))
                    nc.tensor.matmul(po_[:, 512:768], lhsT, Wr["p"][:, k, 512:768],
                                     start=(k == 0), stop=(k == KS - 1))
                ot = opool.tile([P, C], bf16, tag="ot", name="ot")
                nc.vector.tensor_tensor(ot[:], po_[:, 0:768], bpB[:], op=ALU.add)
                nc.scalar.dma_start(y_d[ts_ * P:(ts_ + 1) * P, :], ot[:])

        if "unroll" in variant:
            # straight-line repetition for TimelineSim steady-state analysis
            import re as _re2
            for _ in range(int(_re2.search(r"unroll(\d+)", variant).group(1))):
                body()
        elif reps == 1:
            body()
        else:
            import concourse.mybir as _mb
            with tc.For_i(0, reps, 1, hint_engines=tuple(_mb.ALL_ENGINES)):
                body()

    nc.compile()
    return nc


class _Runner:
    """Compile once, run many times on the 8 axon-tunneled cores via PJRT."""

    def __init__(self, nc, n_cores):
        import jax
        import concourse.mybir as mybir
        from jax.sharding import Mesh, PartitionSpec
        from jax.experimental.shard_map import shard_map
        from concourse.bass2jax import (
            _bass_exec_p, install_neuronx_cc_hook, partition_id_tensor)

        install_neuronx_cc_hook()
        self.jax = jax
        self.n_cores = n_cores
        partition_name = nc.partition_id_tensor.name if nc.partition_id_tensor else None
        in_names, out_names, out_avals, zero_outs = [], [], [], []
        for alloc in nc.m.functions[0].allocations:
            if not isinstance(alloc, mybir.MemoryLocationSet):
                continue
            name = alloc.memorylocations[0].name
            if alloc.kind == "ExternalInput":
                if name != partition_name:
                    in_names.append(name)
            elif alloc.kind == "ExternalOutput":
                shape = tuple(alloc.tensor_shape)
                dtype = mybir.dt.np(alloc.dtype)
                out_names.append(name)
                out_avals.append(jax.core.ShapedArray(shape, dtype))
                zero_outs.append(np.zeros(shape, dtype))
        self.in_names, self.out_names = in_names, out_names
        self.zero_outs = zero_outs
        all_in = list(in_names) + list(out_names)
        if partition_name is not None:
            all_in.append(partition_name)

        def _body(*args):
            operands = list(args)
            if partition_name is not None:
                operands.append(partition_id_tensor())
            return tuple(_bass_exec_p.bind(
                *operands, out_avals=tuple(out_avals), in_names=tuple(all_in),
                out_names=tuple(out_names), lowering_input_output_aliases=(),
                sim_require_finite=True, sim_require_nnan=True, nc=nc))

        # salt the jit name with the BIR content hash so any name+shape-keyed
        # executable cache cannot serve a stale NEFF for a changed kernel
        import hashlib
        _body.__name__ = _body.__qualname__ = (
            f"body_{hashlib.sha256(nc.to_json_bytes()).hexdigest()[:12]}")

        devices = jax.devices()[:n_cores]
        self.mesh = Mesh(np.asarray(devices), ("core",))
        spec = PartitionSpec("core")
        self.fn = jax.jit(
            shard_map(_body, mesh=self.mesh,
                      in_specs=(spec,) * (len(in_names) + len(out_names)),
                      out_specs=(spec,) * len(out_names), check_rep=False),
            keep_unused=True)

    def stage(self, in_maps):
        import jax
        from jax.sharding import PartitionSpec
        concat = [
            np.concatenate([np.asarray(in_maps[c][n]) for c in range(self.n_cores)], axis=0)
            for n in self.in_names
        ] + [np.concatenate([z] * self.n_cores, axis=0) for z in self.zero_outs]
        sharding = jax.sharding.NamedSharding(self.mesh, PartitionSpec("core"))
        return [jax.device_put(a, sharding) for a in concat]

    def run(self, staged):
        outs = self.fn(*staged)
        self.jax.block_until_ready(outs)
        return outs

    def run_to_maps(self, staged):
        outs = self.run(staged)
        res = []
        for c in range(self.n_cores):
            m = {}
            for i, n in enumerate(self.out_names):
                g = np.asarray(outs[i])
                per = g.shape[0] // self.n_cores
                m[n] = g[c * per:(c + 1) * per]
            res.append(m)
        return res


def get_runner(reps: int = 1, phases: int = 4, variant: str = "full"):
    key = (reps, phases, variant)
    if key not in _RUNNER_CACHE:
        nc = build_nc(reps, phases, variant)
        _RUNNER_CACHE[key] = _Runner(nc, N_CORES)
    return _RUNNER_CACHE[key]


def make_in_maps(x, Wq, bq, Wk, bk, Wv, bv, Wp, bp):
    import ml_dtypes
    bf16 = ml_dtypes.bfloat16
    x = np.asarray(x, dtype=np.float32)
    weights = {
        "Wq": np.asarray(Wq, bf16), "Wk": np.asarray(Wk, bf16),
        "Wv": np.asarray(Wv, bf16), "Wp": np.asarray(Wp, bf16),
    }
    bqT = np.ascontiguousarray(np.asarray(bq, np.float32).reshape(KS, P).T)
    bkT = np.ascontiguousarray(np.asarray(bk, np.float32).reshape(KS, P).T)
    bvB = np.ascontiguousarray(np.broadcast_to(np.asarray(bv, np.float32), (P, C)))
    bpB = np.ascontiguousarray(np.broadcast_to(np.asarray(bp, np.float32), (P, C)))
    in_maps = []
    for b in range(B):
        in_maps.append({
            "xT": np.ascontiguousarray(x[b].T.astype(bf16)),
            "Wq": weights["Wq"], "Wk": weights["Wk"],
            "Wv": weights["Wv"], "Wp": weights["Wp"],
            "bqT": bqT, "bkT": bkT, "bvB": bvB, "bpB": bpB,
        })
    return in_maps


def kernel(x, Wq, bq, Wk, bk, Wv, bv, Wp, bp):
    runner = get_runner(reps=1)
    in_maps = make_in_maps(x, Wq, bq, Wk, bk, Wv, bv, Wp, bp)
    staged = runner.stage(in_maps)
    res = runner.run_to_maps(staged)
    return np.stack([res[b]["y"] for b in range(B)], axis=0).astype(np.float32)
